# revision 7
# baseline (speedup 1.0000x reference)
"""Sliding-window causal self-attention with RoPE on 8 Trainium2 NeuronCores.

Problem: B=2, S=2048, D=1024, H=16, HD=64, WINDOW=256, fp32.
Sharding: 2 (batch) x 4 (head-groups of 4 heads). Each core computes its
head-group's QKV projections, RoPE, windowed attention, and a partial output
projection (y_g @ Wo_g.T); the host sums the 4 partials per batch.

All matmuls/activations in float16 (tolerance is 2e-2 global-normalized;
fp16 lands ~5e-4). PSUM accumulation is fp32 throughout.

v2 structure (single dense pipeline, PE never idles):
  - PE warmup matmuls at t=0 (HAM at 8/8 when real work arrives).
  - Per 512-col chunk: q-pass | k-pass | v-pass sharing 2 PSUM banks
    (tags acc0/acc1); score/AV units interleaved between passes.
  - RoPE: PSUM->SBUF copy (scalar), pair-swap via ONE stream_shuffle
    (sign folded into the sin table), 3 elementwise ops (DVE/pool split).
  - Scores: [128,1024] f32 PSUM tiles, bufs=2 (4 banks) -> exp never
    blocks the next unit's matmuls. Exp on scalar, 0/1 triangle masks
    alternating DVE/gpsimd.
  - AV: avps bufs=2 open for the whole kernel; reciprocal_approx_fast
    reads the denominator row straight from PSUM; gpsimd broadcast;
    yT in per-(th,qq) tiles so p3 stiles unblock early.
  - p3: all stiles post-phase (ps3 bufs=2 reuses the projection banks),
    interleaved with trailing scores/AV; one 256KB store per stile.
"""
import sys

for _p in ("/opt/trn_rl_repo", "/root/.axon_site/_ro/trn_rl_repo"):
    if _p not in sys.path:
        sys.path.append(_p)

import numpy as np
import concourse.bacc as bacc
import concourse.mybir as mybir
from concourse.tile import TileContext
from concourse.bass_utils import run_bass_kernel_spmd

F32 = mybir.dt.float32
F16 = mybir.dt.float16
AF = mybir.ActivationFunctionType

B, S, D = 2, 2048, 1024
H, HD = 16, 64
WINDOW = 256
THETA = 10000.0
SCALING = 1.0

HG = 4                      # head-groups (cores per batch)
HPG = H // HG               # heads per group = 4
GD = HPG * HD               # group out width = 256
NKT = D // 128              # 8 contraction tiles
NKB = S // 128              # 16 key blocks
NSC = 4                     # 512-wide s-chunks
SCALE = 1.0 / float(np.sqrt(HD))

SHUF_SWAP = [i ^ 1 for i in range(32)]   # pair-swap within 32 partitions

_CACHE = {}

# key blocks whose full query window is projected once chunk sc is done
READY = {0: [0, 1], 1: [2, 3, 4, 5], 2: [6, 7, 8, 9], 3: [10, 11, 12, 13, 14, 15]}


def _build():
    nc = bacc.Bacc(target_bir_lowering=False, trn_type="TRN2")

    xT = nc.dram_tensor("xT", [128, NKT * S], F16, kind="ExternalInput")
    wq = nc.dram_tensor("wq", [128, NKT * GD], F16, kind="ExternalInput")
    wk = nc.dram_tensor("wk", [128, NKT * GD], F16, kind="ExternalInput")
    wv = nc.dram_tensor("wv", [128, NKT * GD], F16, kind="ExternalInput")
    wo = nc.dram_tensor("wo", [128, GD // 128 * D], F16, kind="ExternalInput")
    cos2 = nc.dram_tensor("cos2", [128, S], F16, kind="ExternalInput")
    sin2 = nc.dram_tensor("sin2", [128, S], F16, kind="ExternalInput")
    mask4 = nc.dram_tensor("mask4", [128, 512], F16, kind="ExternalInput")
    out = nc.dram_tensor("out", [S, D], F16, kind="ExternalOutput")

    xTv = xT.ap().rearrange("p (k s) -> p k s", s=S)

    with TileContext(nc) as tc:
        with tc.tile_pool(name="const", bufs=1) as cpool, \
             tc.tile_pool(name="persist", bufs=1) as ppool:
            wqa = cpool.tile([128, 4, GD], F16)
            wqb = cpool.tile([128, 4, GD], F16)
            wk_sb = cpool.tile([128, NKT, GD], F16)
            wv_sb = cpool.tile([128, NKT, GD], F16)
            wo_sb = cpool.tile([128, GD // 128, D], F16)
            cos_sb = cpool.tile([128, S], F16)
            sin_sb = cpool.tile([128, S], F16)
            mask_sb = cpool.tile([128, 2, 2, 128], F16)
            warm = cpool.tile([128, 512], F16)
            nc.vector.memset(warm[:], 0.0)

            half = NKT * GD // 2
            # sync (SP HWDGE ring): critical path for first matmuls
            nc.sync.dma_start(wqa[:].rearrange("p a b -> p (a b)"),
                              wq.ap()[:, 0:half])
            nc.sync.dma_start(wqb[:].rearrange("p a b -> p (a b)"),
                              wq.ap()[:, half:])
            # chunk-0 x tiles, per-kt for fine-grained readiness
            xt0 = [cpool.tile([128, 512], F16, name=f"x0_{kt}")
                   for kt in range(NKT)]
            for kt in range(NKT):
                nc.sync.dma_start(xt0[kt][:], xTv[:, kt, 0:512])
            # scalar (ACT HWDGE ring): k/v weights + tables
            nc.scalar.dma_start(wk_sb[:].rearrange("p a b -> p (a b)"), wk.ap())
            nc.scalar.dma_start(wv_sb[:].rearrange("p a b -> p (a b)"), wv.ap())
            nc.scalar.dma_start(cos_sb[:], cos2[:])
            nc.scalar.dma_start(sin_sb[:], sin2[:])
            nc.scalar.dma_start(
                mask_sb[:].rearrange("p a b c -> p (a b c)"), mask4.ap())
            # gpsimd (SWDGE): bulk x chunks 1-3 + wo
            xtc = [None] + [cpool.tile([128, NKT, 512], F16, name=f"xc{sc}")
                            for sc in range(1, NSC)]
            for sc in range(1, NSC):
                nc.gpsimd.dma_start(xtc[sc][:],
                                    xTv[:, :, sc * 512:(sc + 1) * 512])
            nc.gpsimd.dma_start(
                wo_sb[:].rearrange("p a b -> p (a b)"), wo.ap())

            def xt(sc, kt):
                return xt0[kt][:] if sc == 0 else xtc[sc][:, kt, :]

            # persistent activations; denominator ones column via memset
            v_sb = [ppool.tile([128, 4 * HPG * 65], F16, name=f"v{sc}")
                    for sc in range(NSC)]
            for sc in range(NSC):
                nc.gpsimd.memset(
                    v_sb[sc][:].rearrange("p (g c) -> p g c", c=65)[:, :, 64],
                    1.0)

            qfc = [[ppool.tile([128, 512], F16, name=f"qf{t}_{c}")
                    for c in range(NSC)] for t in range(2)]
            kfc = [[ppool.tile([128, 512], F16, name=f"kf{t}_{c}")
                    for c in range(NSC)] for t in range(2)]
            yTq = [[ppool.tile([128, 512], F16, name=f"yT{t}_{g}")
                    for g in range(4)] for t in range(2)]

            with tc.tile_pool(name="attn", bufs=1) as apool, \
                 tc.tile_pool(name="smalls", bufs=4) as spool, \
                 tc.tile_pool(name="rope", bufs=3) as rawpool, \
                 tc.tile_pool(name="p3sb", bufs=4) as opool:
                attns = [[apool.tile([128, 768], F16, name=f"attn{th}_{kb}")
                          for kb in range(NKB)] for th in range(2)]
                u_cnt = [0]

                with tc.tile_pool(name="scps", bufs=2, space="PSUM") as scps, \
                     tc.tile_pool(name="avps", bufs=2, space="PSUM") as avps:

                    def emit_score(th, kb):
                        q0 = kb * 128
                        n = min(384, S - q0)
                        sc_t = scps.tile([128, 1024], F32, tag="sc",
                                         name=f"sc{th}_{kb}")
                        c0, off0 = q0 // 512, q0 % 512
                        w0 = min(n, 512 - off0)
                        pieces = [(c0, off0, 0, w0)]
                        if w0 < n:
                            pieces.append((c0 + 1, 0, w0, n - w0))
                        for i in range(2):
                            ph = 64 * i
                            for (cc, coff, aoff, w) in pieces:
                                nc.tensor.matmul(
                                    sc_t[:, i * 512 + aoff:i * 512 + aoff + w],
                                    kfc[th][c0][ph:ph + 64, off0:off0 + 128],
                                    qfc[th][cc][ph:ph + 64, coff:coff + w],
                                    start=True, stop=True)
                        scv = sc_t[:].rearrange("p (g c) -> p g c", g=2)
                        at = attns[th][kb]
                        atv = at[:].rearrange("p (g c) -> p g c", g=2)
                        nc.scalar.activation(atv[:, :, 0:n], scv[:, :, 0:n],
                                             AF.Exp, scale=SCALE)
                        # multiplicative 0/1 mask on the triangle blocks
                        eng = nc.gpsimd if u_cnt[0] % 8 >= 5 else nc.vector
                        u_cnt[0] += 1
                        if n == 384:
                            at4 = at[:].rearrange("p (g b c) -> p g b c",
                                                  g=2, b=3)[:, :, 0::2, :]
                            eng.tensor_mul(at4, at4, mask_sb[:])
                        else:
                            eng.tensor_mul(atv[:, :, 0:128], atv[:, :, 0:128],
                                           mask_sb[:, :, 0, :])

                    def emit_av(th, i, qq):
                        h = 2 * th + i
                        ph = 64 * i
                        acc = avps.tile([65, 512], F32, tag="av",
                                        name=f"av{th}_{i}_{qq}")
                        first = True
                        last = []
                        for j2 in range(2):      # 256-q halves
                            m = 2 * qq + j2
                            qb0 = 2 * m
                            mms = [(qb0, 0, 0, 256)]
                            if m >= 1:
                                mms.append((qb0 - 2, 0, 256, 128))
                                mms.append((qb0 - 1, 0, 128, 256))
                            mms.append((qb0 + 1, 128, 0, 128))
                            for ii, (kb, jo, ao, w) in enumerate(mms):
                                wdt = min(w, S - kb * 128 - ao)
                                nc.tensor.matmul(
                                    acc[:, j2 * 256 + jo:j2 * 256 + jo + wdt],
                                    v_sb[kb // 4][:, ((kb % 4) * HPG + h) * 65:
                                                  ((kb % 4) * HPG + h) * 65 + 65],
                                    attns[th][kb][:, i * 384 + ao:
                                                  i * 384 + ao + wdt],
                                    start=first,
                                    stop=(j2 == 1 and ii == len(mms) - 1))
                                first = False
                        den = spool.tile([1, 512], F32, tag="den")
                        nc.scalar.copy(den[:], acc[64:65, :])
                        rc0 = spool.tile([1, 512], F32, tag="rc0")
                        nc.vector.reciprocal_approx_fast(
                            out=rc0[:], in_=den[:])
                        rbs = spool.tile([64, 512], F32, tag="rbs")
                        nc.gpsimd.partition_broadcast(rbs[:], rc0[:])
                        nc.vector.tensor_mul(
                            yTq[th][qq][ph:ph + 64, :], acc[0:64, :], rbs[:])

                    def rope_evac(acc, dst, s0, alt):
                        raw = rawpool.tile([128, 512], F16, tag="raw")
                        nc.scalar.copy(raw[:], acc[:])
                        rot = rawpool.tile([128, 512], F16, tag="rot")
                        nc.vector.stream_shuffle(rot[:], raw[:], SHUF_SWAP)
                        t1 = rawpool.tile([128, 512], F16, tag="t1")
                        nc.vector.tensor_mul(t1[:], rot[:],
                                             sin_sb[:, s0:s0 + 512])
                        t2 = rawpool.tile([128, 512], F16, tag="t2")
                        eng = nc.gpsimd if alt else nc.vector
                        eng.tensor_mul(t2[:], raw[:], cos_sb[:, s0:s0 + 512])
                        nc.vector.tensor_add(dst[:], t1[:], t2[:])

                    with tc.tile_pool(name="p1acc", bufs=1,
                                      space="PSUM") as pps:
                        # PE warmup: ~3.4us of dummy matmuls so HAM reaches
                        # 8/8 by the time the first x/w tiles land
                        wacc = pps.tile([128, 512], F32, tag="acc0",
                                        name="warmacc")
                        for _ in range(8):
                            nc.tensor.matmul(wacc[:], warm[0:128, 0:128],
                                             warm[:], start=True, stop=True)

                        def units(lst):
                            for u in lst:
                                if u[0] == "s":
                                    emit_score(u[1], u[2])
                                else:
                                    emit_av(u[1], u[2], u[3])

                        UA = {0: [], 1: [("s", 0, 0), ("s", 1, 0)],
                              2: [("s", 0, 2), ("s", 1, 2)],
                              3: [("s", 0, 6), ("s", 1, 6)]}
                        UB = {0: [], 1: [("s", 0, 1), ("s", 1, 1)],
                              2: [("s", 0, 3), ("s", 1, 3)],
                              3: [("s", 0, 7), ("s", 1, 7)]}
                        UC = {0: [], 1: [],
                              2: [("s", 0, 4), ("s", 1, 4),
                                  ("s", 0, 5), ("s", 1, 5),
                                  ("a", 0, 0, 0), ("a", 0, 1, 0),
                                  ("a", 1, 0, 0), ("a", 1, 1, 0)],
                              3: [("s", 0, 8), ("s", 1, 8),
                                  ("s", 0, 9), ("s", 1, 9),
                                  ("a", 0, 0, 1), ("a", 0, 1, 1),
                                  ("a", 1, 0, 1), ("a", 1, 1, 1)]}

                        for sc in range(NSC):
                            s0 = sc * 512
                            # ---- q-pass ----
                            aq = [pps.tile([128, 512], F32, tag=f"acc{t}",
                                           name=f"q{sc}_{t}")
                                  for t in range(2)]
                            for kt in range(NKT):
                                w_t = wqa if kt < 4 else wqb
                                for t in range(2):
                                    nc.tensor.matmul(
                                        aq[t][:],
                                        w_t[:, kt % 4, t * 128:t * 128 + 128],
                                        xt(sc, kt),
                                        start=(kt == 0), stop=(kt == NKT - 1))
                            rope_evac(aq[0], qfc[0][sc], s0, False)
                            rope_evac(aq[1], qfc[1][sc], s0, True)
                            units(UA[sc])
                            # ---- k-pass ----
                            ak = [pps.tile([128, 512], F32, tag=f"acc{t}",
                                           name=f"k{sc}_{t}")
                                  for t in range(2)]
                            for kt in range(NKT):
                                for t in range(2):
                                    nc.tensor.matmul(
                                        ak[t][:],
                                        wk_sb[:, kt, t * 128:t * 128 + 128],
                                        xt(sc, kt),
                                        start=(kt == 0), stop=(kt == NKT - 1))
                            rope_evac(ak[0], kfc[0][sc], s0, False)
                            rope_evac(ak[1], kfc[1][sc], s0, True)
                            units(UB[sc])
                            # ---- v-pass (x stationary, wv moving) ----
                            av_ = [pps.tile([128, 512], F32, tag=f"acc{j}",
                                            name=f"v{sc}_{j}")
                                   for j in range(2)]
                            for kt in range(NKT):
                                st, sp = (kt == 0), (kt == NKT - 1)
                                for j in range(2):
                                    for jj in range(2):
                                        sb = 2 * j + jj
                                        nc.tensor.matmul(
                                            av_[j][:, jj * 256:(jj + 1) * 256],
                                            xt(sc, kt)[:, sb * 128:
                                                       (sb + 1) * 128],
                                            wv_sb[:, kt, 0:256],
                                            start=(st and jj == 0),
                                            stop=(sp and jj == 1))
                            # v evacuation into the 65-stride layout
                            for j in range(2):
                                for jj in range(2):
                                    kbl = 2 * j + jj
                                    dstv = v_sb[sc][:, kbl * HPG * 65:
                                                    (kbl + 1) * HPG * 65]
                                    src = av_[j][:, jj * 256:(jj + 1) * 256] \
                                        .rearrange("p (g c) -> p g c", c=64)
                                    dv = dstv.rearrange("p (g c) -> p g c",
                                                        c=65)[:, :, 0:64]
                                    nc.vector.tensor_copy(dv, src)
                            units(UC[sc])

                    # ---- post phase: trailing scores + AV qq2/3 + p3 ----
                    with tc.tile_pool(name="p3ps", bufs=2,
                                      space="PSUM") as ps3:

                        def emit_p3(stile):
                            r0 = stile * 128
                            g, o = stile // 4, (stile % 4) * 128
                            ot = opool.tile([128, D], F16, tag="ot")
                            for dc in range(2):
                                oacc = ps3.tile([128, 512], F32, tag="oacc")
                                for ct in range(2):
                                    nc.tensor.matmul(
                                        oacc[:], yTq[ct][g][:, o:o + 128],
                                        wo_sb[:, ct, dc * 512:(dc + 1) * 512],
                                        start=(ct == 0), stop=(ct == 1))
                                if dc == 0:
                                    nc.scalar.copy(ot[:, 0:512], oacc[:])
                                else:
                                    nc.vector.tensor_copy(
                                        ot[:, 512:1024], oacc[:])
                            nc.sync.dma_start(out.ap()[r0:r0 + 128, :], ot[:])

                        for kb in (10, 11, 12, 13):
                            for th in range(2):
                                emit_score(th, kb)
                        emit_p3(0)
                        emit_p3(1)
                        for kb in (14, 15):
                            for th in range(2):
                                emit_score(th, kb)
                        emit_av(0, 0, 2)
                        emit_av(0, 1, 2)
                        emit_p3(2)
                        emit_av(1, 0, 2)
                        emit_av(1, 1, 2)
                        emit_p3(3)
                        emit_av(0, 0, 3)
                        emit_av(0, 1, 3)
                        emit_p3(4)
                        emit_av(1, 0, 3)
                        emit_av(1, 1, 3)
                        emit_p3(5)
                        for stile in range(6, 16):
                            emit_p3(stile)

    nc.finalize()
    return nc


def _rope_tables():
    inv_freq = 1.0 / (THETA ** (np.arange(0, HD, 2, dtype=np.float64) / HD))
    t = np.arange(S, dtype=np.float64) / max(SCALING, 1e-6)
    freqs = np.outer(t, inv_freq)                      # [S, HD/2]
    emb = np.concatenate((freqs, freqs), axis=-1)      # [S, HD]
    return np.cos(emb), np.sin(emb)


def _swz(w):
    # [kt*128, X] -> [128, kt*X] partition-major contiguous
    kt = w.shape[0] // 128
    return np.ascontiguousarray(
        w.reshape(kt, 128, w.shape[1]).transpose(1, 0, 2).reshape(128, -1))


def _host_prep(x, Wq, Wk, Wv, Wo):
    cos, sin = _rope_tables()
    cosT2 = np.ascontiguousarray(np.tile(cos.T, (2, 1))).astype(np.float16)
    sinT2 = np.ascontiguousarray(np.tile(sin.T, (2, 1))).astype(np.float16)
    # fold the rotate-half signs into the sin table: rot[2i] = -raw[2i+1],
    # rot[2i+1] = +raw[2i]; the shuffle moves values unsigned
    sinT2[0::2, :] *= -1.0

    # mask4 [128 key, (i=2, block=2, 128 col)]: block 0 = causal triangle of
    # the kb-aligned window block, block 1 = far-window triangle (col+256)
    cc = np.arange(128)[None, :]
    kk = np.arange(128)[:, None]
    mb0 = (cc >= kk).astype(np.float16)        # [128, 128]
    mb1 = (cc < kk).astype(np.float16)
    mi = np.concatenate([mb0, mb1], axis=1)    # [128, 256]
    m4 = np.ascontiguousarray(np.concatenate([mi, mi], axis=1))  # [128, 512]

    in_maps = []
    for c in range(8):
        b, g = c // HG, c % HG
        gsl = slice(g * GD, (g + 1) * GD)
        in_maps.append({
            "xT": _swz(x[b].T.astype(np.float16).reshape(D, S)),
            "wq": _swz(Wq[gsl, :].T).astype(np.float16),
            "wk": _swz(Wk[gsl, :].T).astype(np.float16),
            "wv": _swz(Wv[gsl, :].T).astype(np.float16),
            "wo": _swz(Wo[:, gsl].T).astype(np.float16),
            "cos2": cosT2, "sin2": sinT2, "mask4": m4,
        })
    return in_maps


def _run(inputs, trace=False, **kw):
    if "nc" not in _CACHE:
        _CACHE["nc"] = _build()
    in_maps = _host_prep(inputs["x"], inputs["Wq"], inputs["Wk"],
                         inputs["Wv"], inputs["Wo"])
    return run_bass_kernel_spmd(_CACHE["nc"], in_maps, list(range(8)),
                                trace=trace, **kw)


def kernel(x, Wq, Wk, Wv, Wo):
    res = _run({"x": x, "Wq": Wq, "Wk": Wk, "Wv": Wv, "Wo": Wo})
    out = np.zeros((B, S, D), dtype=np.float32)
    for c in range(8):
        out[c // HG] += res.results[c]["out"].astype(np.float32)
    return out


# revision 9
# speedup vs baseline: 1.0962x; 1.0962x over previous
"""Sliding-window causal self-attention with RoPE on 8 Trainium2 NeuronCores.

Problem: B=2, S=2048, D=1024, H=16, HD=64, WINDOW=256, fp32.
Sharding: 2 (batch) x 4 (head-groups of 4 heads). Each core computes its
head-group's QKV projections, RoPE, windowed attention, and a partial output
projection (y_g @ Wo_g.T); the host sums the 4 partials per batch.

All matmuls/activations in float16 (tolerance is 2e-2 global-normalized;
fp16 lands ~5e-4). PSUM accumulation is fp32 throughout.

v3 structure:
  - PE warmup matmuls at t=0 (HAM at 8/8 when real work arrives); all x
    DMAs on the sync ring in chunk order, weights/tables on scalar ring.
  - Phase A per 512-col chunk: dense 4-acc q/k kt-loop (banks 0-3),
    rope evac (PSUM copy + ONE stream_shuffle pair-swap + 3 elementwise,
    sign folded into the sin table), score units, v-pass reusing banks
    2-3, v evac, more score units. Scores double-buffered ([128,1024]
    f32 x2 = 4 banks) so exp never blocks the next unit's matmuls.
  - Post phase: trailing scores (2-deep), AV with immediate [65,512]
    PSUM->SBUF copy (bank releases without waiting the normalization
    chain: recip -> gpsimd broadcast -> yT mul), p3 stiles interleaved;
    one 256KB store per stile on the sync ring.
"""
import sys

for _p in ("/opt/trn_rl_repo", "/root/.axon_site/_ro/trn_rl_repo"):
    if _p not in sys.path:
        sys.path.append(_p)

import numpy as np
import concourse.bacc as bacc
import concourse.mybir as mybir
from concourse.tile import TileContext
from concourse.bass_utils import run_bass_kernel_spmd

F32 = mybir.dt.float32
F16 = mybir.dt.float16
AF = mybir.ActivationFunctionType

B, S, D = 2, 2048, 1024
H, HD = 16, 64
WINDOW = 256
THETA = 10000.0
SCALING = 1.0

HG = 4                      # head-groups (cores per batch)
HPG = H // HG               # heads per group = 4
GD = HPG * HD               # group out width = 256
NKT = D // 128              # 8 contraction tiles
NKB = S // 128              # 16 key blocks
NSC = 4                     # 512-wide s-chunks
SCALE = 1.0 / float(np.sqrt(HD))

SHUF_SWAP = [i ^ 1 for i in range(32)]   # pair-swap within 32 partitions

_CACHE = {}


def _build():
    nc = bacc.Bacc(target_bir_lowering=False, trn_type="TRN2")

    xT = nc.dram_tensor("xT", [128, NKT * S], F16, kind="ExternalInput")
    wq = nc.dram_tensor("wq", [128, NKT * GD], F16, kind="ExternalInput")
    wk = nc.dram_tensor("wk", [128, NKT * GD], F16, kind="ExternalInput")
    wv = nc.dram_tensor("wv", [128, NKT * GD], F16, kind="ExternalInput")
    wo = nc.dram_tensor("wo", [128, GD // 128 * D], F16, kind="ExternalInput")
    cos2 = nc.dram_tensor("cos2", [128, S], F16, kind="ExternalInput")
    sin2 = nc.dram_tensor("sin2", [128, S], F16, kind="ExternalInput")
    mask4 = nc.dram_tensor("mask4", [128, 512], F16, kind="ExternalInput")
    out = nc.dram_tensor("out", [S, D], F16, kind="ExternalOutput")

    with TileContext(nc) as tc:
        with tc.tile_pool(name="const", bufs=1) as cpool, \
             tc.tile_pool(name="persist", bufs=1) as ppool:
            wqa = cpool.tile([128, 4, GD], F16)
            wqb = cpool.tile([128, 4, GD], F16)
            wk_sb = cpool.tile([128, NKT, GD], F16)
            wv_sb = cpool.tile([128, NKT, GD], F16)
            wo_sb = cpool.tile([128, GD // 128, D], F16)
            cos_sb = cpool.tile([128, S], F16)
            sin_sb = cpool.tile([128, S], F16)
            mask_sb = cpool.tile([128, 2, 2, 128], F16)
            warm = cpool.tile([128, 512], F16)
            nc.vector.memset(warm[:], 0.0)

            half = NKT * GD // 2
            # sync (SP HWDGE ring): wq first, then all x in chunk order
            nc.sync.dma_start(wqa[:].rearrange("p a b -> p (a b)"),
                              wq.ap()[:, 0:half])
            nc.sync.dma_start(wqb[:].rearrange("p a b -> p (a b)"),
                              wq.ap()[:, half:])
            xt = [[cpool.tile([128, 512], F16, name=f"x{kt}_{sc}")
                   for sc in range(NSC)] for kt in range(NKT)]
            for sc in range(NSC):
                for kt in range(NKT):
                    nc.sync.dma_start(
                        xt[kt][sc][:],
                        xT.ap()[:, kt * S + sc * 512:kt * S + sc * 512 + 512])
            # scalar (ACT HWDGE ring): k/v weights + tables + wo
            nc.scalar.dma_start(wk_sb[:].rearrange("p a b -> p (a b)"), wk.ap())
            nc.scalar.dma_start(wv_sb[:].rearrange("p a b -> p (a b)"), wv.ap())
            nc.scalar.dma_start(cos_sb[:], cos2[:])
            nc.scalar.dma_start(sin_sb[:], sin2[:])
            nc.scalar.dma_start(
                mask_sb[:].rearrange("p a b c -> p (a b c)"), mask4.ap())
            nc.scalar.dma_start(
                wo_sb[:].rearrange("p a b -> p (a b)"), wo.ap())

            # persistent activations; denominator ones column via memset
            v_sb = [ppool.tile([128, 4 * HPG * 65], F16, name=f"v{sc}")
                    for sc in range(NSC)]
            for sc in range(NSC):
                nc.gpsimd.memset(
                    v_sb[sc][:].rearrange("p (g c) -> p g c", c=65)[:, :, 64],
                    1.0)

            qfc = [[ppool.tile([128, 512], F16, name=f"qf{t}_{c}")
                    for c in range(NSC)] for t in range(2)]
            kfc = [[ppool.tile([128, 512], F16, name=f"kf{t}_{c}")
                    for c in range(NSC)] for t in range(2)]
            yTq = [[ppool.tile([128, 512], F16, name=f"yT{t}_{g}")
                    for g in range(4)] for t in range(2)]

            with tc.tile_pool(name="attn", bufs=1) as apool, \
                 tc.tile_pool(name="smalls", bufs=4) as spool, \
                 tc.tile_pool(name="rope", bufs=3) as rawpool, \
                 tc.tile_pool(name="p3sb", bufs=4) as opool:
                attns = [[apool.tile([128, 768], F16, name=f"attn{th}_{kb}")
                          for kb in range(NKB)] for th in range(2)]
                u_cnt = [0]

                def emit_score(th, kb, scps):
                    q0 = kb * 128
                    n = min(384, S - q0)
                    sc_t = scps.tile([128, 1024], F32, tag="sc",
                                     name=f"sc{th}_{kb}")
                    c0, off0 = q0 // 512, q0 % 512
                    w0 = min(n, 512 - off0)
                    pieces = [(c0, off0, 0, w0)]
                    if w0 < n:
                        pieces.append((c0 + 1, 0, w0, n - w0))
                    for i in range(2):
                        ph = 64 * i
                        for (cc, coff, aoff, w) in pieces:
                            nc.tensor.matmul(
                                sc_t[:, i * 512 + aoff:i * 512 + aoff + w],
                                kfc[th][c0][ph:ph + 64, off0:off0 + 128],
                                qfc[th][cc][ph:ph + 64, coff:coff + w],
                                start=True, stop=True)
                    scv = sc_t[:].rearrange("p (g c) -> p g c", g=2)
                    at = attns[th][kb]
                    atv = at[:].rearrange("p (g c) -> p g c", g=2)
                    nc.scalar.activation(atv[:, :, 0:n], scv[:, :, 0:n],
                                         AF.Exp, scale=SCALE)
                    # multiplicative 0/1 mask on the triangle blocks
                    eng = nc.gpsimd if u_cnt[0] % 8 >= 5 else nc.vector
                    u_cnt[0] += 1
                    if n == 384:
                        at4 = at[:].rearrange("p (g b c) -> p g b c",
                                              g=2, b=3)[:, :, 0::2, :]
                        eng.tensor_mul(at4, at4, mask_sb[:])
                    else:
                        eng.tensor_mul(atv[:, :, 0:128], atv[:, :, 0:128],
                                       mask_sb[:, :, 0, :])

                def rope_evac(acc, dst, s0, alt):
                    raw = rawpool.tile([128, 512], F16, tag="raw")
                    nc.scalar.copy(raw[:], acc[:])
                    rot = rawpool.tile([128, 512], F16, tag="rot")
                    nc.vector.stream_shuffle(rot[:], raw[:], SHUF_SWAP)
                    t1 = rawpool.tile([128, 512], F16, tag="t1")
                    nc.vector.tensor_mul(t1[:], rot[:],
                                         sin_sb[:, s0:s0 + 512])
                    t2 = rawpool.tile([128, 512], F16, tag="t2")
                    eng = nc.gpsimd if alt else nc.vector
                    eng.tensor_mul(t2[:], raw[:], cos_sb[:, s0:s0 + 512])
                    nc.vector.tensor_add(dst[:], t1[:], t2[:])

                # ---------------- phase A ----------------
                with tc.tile_pool(name="scpsA", bufs=2, space="PSUM") as scA:
                    with tc.tile_pool(name="p1acc", bufs=1,
                                      space="PSUM") as pps:
                        # PE warmup: ~3.4us of dummy matmuls so HAM reaches
                        # 8/8 by the time the first x/w tiles land
                        wacc = pps.tile([128, 512], F32, tag="acc0",
                                        name="warmacc")
                        for _ in range(8):
                            nc.tensor.matmul(wacc[:], warm[0:128, 0:128],
                                             warm[:], start=True, stop=True)

                        UM = {0: [], 1: [(0, 0), (1, 0)],
                              2: [(0, 2), (1, 2), (0, 3), (1, 3)],
                              3: [(0, 6), (1, 6), (0, 7), (1, 7)]}
                        UE = {0: [], 1: [(0, 1), (1, 1)],
                              2: [(0, 4), (1, 4), (0, 5), (1, 5)],
                              3: [(0, 8), (1, 8), (0, 9), (1, 9)]}

                        for sc in range(NSC):
                            s0 = sc * 512
                            accs = [pps.tile([128, 512], F32, tag=f"acc{t}",
                                             name=f"qk{sc}_{t}")
                                    for t in range(4)]
                            wsel = [(0, 0), (0, 128), (1, 0), (1, 128)]
                            for kt in range(NKT):
                                for t, (isk, off) in enumerate(wsel):
                                    if isk:
                                        w_t = wk_sb[:, kt, off:off + 128]
                                    elif kt < 4:
                                        w_t = wqa[:, kt, off:off + 128]
                                    else:
                                        w_t = wqb[:, kt - 4, off:off + 128]
                                    nc.tensor.matmul(
                                        accs[t][:], w_t, xt[kt][sc][:],
                                        start=(kt == 0), stop=(kt == NKT - 1))
                            rope_evac(accs[0], qfc[0][sc], s0, False)
                            rope_evac(accs[1], qfc[1][sc], s0, True)
                            rope_evac(accs[2], kfc[0][sc], s0, False)
                            rope_evac(accs[3], kfc[1][sc], s0, True)
                            for (th, kb) in UM[sc]:
                                emit_score(th, kb, scA)
                            # v-pass reuses banks 2-3 (k accs, freed by the
                            # rope PSUM copies)
                            av_ = [pps.tile([128, 512], F32, tag=f"acc{j+2}",
                                            name=f"v{sc}_{j}")
                                   for j in range(2)]
                            for kt in range(NKT):
                                st, sp = (kt == 0), (kt == NKT - 1)
                                for j in range(2):
                                    for jj in range(2):
                                        sb = 2 * j + jj
                                        nc.tensor.matmul(
                                            av_[j][:, jj * 256:(jj + 1) * 256],
                                            xt[kt][sc][:, sb * 128:
                                                       (sb + 1) * 128],
                                            wv_sb[:, kt, 0:256],
                                            start=(st and jj == 0),
                                            stop=(sp and jj == 1))
                            for j in range(2):
                                for jj in range(2):
                                    kbl = 2 * j + jj
                                    dstv = v_sb[sc][:, kbl * HPG * 65:
                                                    (kbl + 1) * HPG * 65]
                                    src = av_[j][:, jj * 256:(jj + 1) * 256] \
                                        .rearrange("p (g c) -> p g c", c=64)
                                    dv = dstv.rearrange("p (g c) -> p g c",
                                                        c=65)[:, :, 0:64]
                                    nc.vector.tensor_copy(dv, src)
                            for (th, kb) in UE[sc]:
                                emit_score(th, kb, scA)

                # ---------------- post phase ----------------
                with tc.tile_pool(name="scpsB", bufs=2, space="PSUM") as scB, \
                     tc.tile_pool(name="avps", bufs=2, space="PSUM") as avps, \
                     tc.tile_pool(name="p3ps", bufs=2, space="PSUM") as ps3:

                    def emit_av(th, i, qq):
                        h = 2 * th + i
                        ph = 64 * i
                        acc = avps.tile([65, 512], F32, tag="av",
                                        name=f"av{th}_{i}_{qq}")
                        first = True
                        for j2 in range(2):      # 256-q halves
                            m = 2 * qq + j2
                            qb0 = 2 * m
                            mms = [(qb0, 0, 0, 256)]
                            if m >= 1:
                                mms.append((qb0 - 2, 0, 256, 128))
                                mms.append((qb0 - 1, 0, 128, 256))
                            mms.append((qb0 + 1, 128, 0, 128))
                            for ii, (kb, jo, ao, w) in enumerate(mms):
                                wdt = min(w, S - kb * 128 - ao)
                                nc.tensor.matmul(
                                    acc[:, j2 * 256 + jo:j2 * 256 + jo + wdt],
                                    v_sb[kb // 4][:, ((kb % 4) * HPG + h) * 65:
                                                  ((kb % 4) * HPG + h) * 65 + 65],
                                    attns[th][kb][:, i * 384 + ao:
                                                  i * 384 + ao + wdt],
                                    start=first,
                                    stop=(j2 == 1 and ii == len(mms) - 1))
                                first = False
                        den = spool.tile([1, 512], F32, tag="den")
                        nc.scalar.copy(den[:], acc[64:65, :])
                        rc0 = spool.tile([1, 512], F32, tag="rc0")
                        nc.vector.reciprocal_approx_fast(
                            out=rc0[:], in_=den[:])
                        rbs = spool.tile([64, 512], F32, tag="rbs")
                        nc.gpsimd.partition_broadcast(rbs[:], rc0[:])
                        nc.vector.tensor_mul(
                            yTq[th][qq][ph:ph + 64, :], acc[0:64, :], rbs[:])

                    def emit_p3(stile):
                        r0 = stile * 128
                        g, o = stile // 4, (stile % 4) * 128
                        ot = opool.tile([128, D], F16, tag="ot")
                        for dc in range(2):
                            oacc = ps3.tile([128, 512], F32, tag="oacc")
                            for ct in range(2):
                                nc.tensor.matmul(
                                    oacc[:], yTq[ct][g][:, o:o + 128],
                                    wo_sb[:, ct, dc * 512:(dc + 1) * 512],
                                    start=(ct == 0), stop=(ct == 1))
                            if dc == 0:
                                nc.scalar.copy(ot[:, 0:512], oacc[:])
                            else:
                                nc.vector.tensor_copy(ot[:, 512:1024], oacc[:])
                        nc.sync.dma_start(out.ap()[r0:r0 + 128, :], ot[:])

                    def trail(kb):
                        for th in range(2):
                            emit_score(th, kb, scB)

                    trail(10)
                    for i in range(2):
                        emit_av(0, i, 0)
                        emit_av(1, i, 0)
                    trail(11)
                    for i in range(2):
                        emit_av(0, i, 1)
                        emit_av(1, i, 1)
                    trail(12)
                    trail(13)
                    emit_p3(0)
                    emit_p3(1)
                    for i in range(2):
                        emit_av(0, i, 2)
                        emit_av(1, i, 2)
                    trail(14)
                    trail(15)
                    emit_p3(2)
                    emit_p3(3)
                    for i in range(2):
                        emit_av(0, i, 3)
                        emit_av(1, i, 3)
                    for stile in range(4, 16):
                        emit_p3(stile)

    nc.finalize()
    return nc


def _rope_tables():
    inv_freq = 1.0 / (THETA ** (np.arange(0, HD, 2, dtype=np.float64) / HD))
    t = np.arange(S, dtype=np.float64) / max(SCALING, 1e-6)
    freqs = np.outer(t, inv_freq)                      # [S, HD/2]
    emb = np.concatenate((freqs, freqs), axis=-1)      # [S, HD]
    return np.cos(emb), np.sin(emb)


def _swz(w):
    # [kt*128, X] -> [128, kt*X] partition-major contiguous
    kt = w.shape[0] // 128
    return np.ascontiguousarray(
        w.reshape(kt, 128, w.shape[1]).transpose(1, 0, 2).reshape(128, -1))


def _host_prep(x, Wq, Wk, Wv, Wo):
    cos, sin = _rope_tables()
    cosT2 = np.ascontiguousarray(np.tile(cos.T, (2, 1))).astype(np.float16)
    sinT2 = np.ascontiguousarray(np.tile(sin.T, (2, 1))).astype(np.float16)
    # fold the rotate-half signs into the sin table: rot[2i] = -raw[2i+1],
    # rot[2i+1] = +raw[2i]; the shuffle moves values unsigned
    sinT2[0::2, :] *= -1.0

    # mask4 [128 key, (i=2, block=2, 128 col)]: block 0 = causal triangle of
    # the kb-aligned window block, block 1 = far-window triangle (col+256)
    cc = np.arange(128)[None, :]
    kk = np.arange(128)[:, None]
    mb0 = (cc >= kk).astype(np.float16)        # [128, 128]
    mb1 = (cc < kk).astype(np.float16)
    mi = np.concatenate([mb0, mb1], axis=1)    # [128, 256]
    m4 = np.ascontiguousarray(np.concatenate([mi, mi], axis=1))  # [128, 512]

    in_maps = []
    for c in range(8):
        b, g = c // HG, c % HG
        gsl = slice(g * GD, (g + 1) * GD)
        in_maps.append({
            "xT": _swz(x[b].T.astype(np.float16).reshape(D, S)),
            "wq": _swz(Wq[gsl, :].T).astype(np.float16),
            "wk": _swz(Wk[gsl, :].T).astype(np.float16),
            "wv": _swz(Wv[gsl, :].T).astype(np.float16),
            "wo": _swz(Wo[:, gsl].T).astype(np.float16),
            "cos2": cosT2, "sin2": sinT2, "mask4": m4,
        })
    return in_maps


def _run(inputs, trace=False, **kw):
    if "nc" not in _CACHE:
        _CACHE["nc"] = _build()
    in_maps = _host_prep(inputs["x"], inputs["Wq"], inputs["Wk"],
                         inputs["Wv"], inputs["Wo"])
    return run_bass_kernel_spmd(_CACHE["nc"], in_maps, list(range(8)),
                                trace=trace, **kw)


def kernel(x, Wq, Wk, Wv, Wo):
    res = _run({"x": x, "Wq": Wq, "Wk": Wk, "Wv": Wv, "Wo": Wo})
    out = np.zeros((B, S, D), dtype=np.float32)
    for c in range(8):
        out[c // HG] += res.results[c]["out"].astype(np.float32)
    return out


# revision 14
# speedup vs baseline: 1.3996x; 1.2768x over previous
"""Sliding-window causal self-attention with RoPE on 8 Trainium2 NeuronCores.

Problem: B=2, S=2048, D=1024, H=16, HD=64, WINDOW=256, fp32.
Sharding: 2 (batch) x 4 (head-groups of 4 heads). Each core computes its
head-group's QKV projections, RoPE, windowed attention, and a partial output
projection (y_g @ Wo_g.T); the host sums the 4 partials per batch.

All matmuls/activations in float16 (tolerance is 2e-2 global-normalized;
fp16 lands ~5e-4). PSUM accumulation is fp32 throughout.

v3 structure:
  - PE warmup matmuls at t=0 (HAM at 8/8 when real work arrives); all x
    DMAs on the sync ring in chunk order, weights/tables on scalar ring.
  - Phase A per 512-col chunk: dense 4-acc q/k kt-loop (banks 0-3),
    rope evac (PSUM copy + ONE stream_shuffle pair-swap + 3 elementwise,
    sign folded into the sin table), score units, v-pass reusing banks
    2-3, v evac, more score units. Scores double-buffered ([128,1024]
    f32 x2 = 4 banks) so exp never blocks the next unit's matmuls.
  - Post phase: trailing scores (2-deep), AV with immediate [65,512]
    PSUM->SBUF copy (bank releases without waiting the normalization
    chain: recip -> gpsimd broadcast -> yT mul), p3 stiles interleaved;
    one 256KB store per stile on the sync ring.
"""
import sys

for _p in ("/opt/trn_rl_repo", "/root/.axon_site/_ro/trn_rl_repo"):
    if _p not in sys.path:
        sys.path.append(_p)

import numpy as np
import concourse.bacc as bacc
import concourse.mybir as mybir
from concourse.tile import TileContext
from concourse.bass_utils import run_bass_kernel_spmd

F32 = mybir.dt.float32
F16 = mybir.dt.float16
AF = mybir.ActivationFunctionType

B, S, D = 2, 2048, 1024
H, HD = 16, 64
WINDOW = 256
THETA = 10000.0
SCALING = 1.0

HG = 4                      # head-groups (cores per batch)
HPG = H // HG               # heads per group = 4
GD = HPG * HD               # group out width = 256
NKT = D // 128              # 8 contraction tiles
NKB = S // 128              # 16 key blocks
NSC = 4                     # 512-wide s-chunks
SCALE = 1.0 / float(np.sqrt(HD))

SHUF_SWAP = [i ^ 1 for i in range(32)]   # pair-swap within 32 partitions

_CACHE = {}


def _build():
    nc = bacc.Bacc(target_bir_lowering=False, trn_type="TRN2")

    xT = nc.dram_tensor("xT", [128, NKT * S], F16, kind="ExternalInput")
    wq = nc.dram_tensor("wq", [128, NKT * GD], F16, kind="ExternalInput")
    wk = nc.dram_tensor("wk", [128, NKT * GD], F16, kind="ExternalInput")
    wv = nc.dram_tensor("wv", [128, NKT * GD], F16, kind="ExternalInput")
    wo = nc.dram_tensor("wo", [128, GD // 128 * D], F16, kind="ExternalInput")
    cos2 = nc.dram_tensor("cos2", [128, S], F16, kind="ExternalInput")
    sin2 = nc.dram_tensor("sin2", [128, S], F16, kind="ExternalInput")
    mask4 = nc.dram_tensor("mask4", [128, 512], F16, kind="ExternalInput")
    out = nc.dram_tensor("out", [S, D], F16, kind="ExternalOutput")

    with TileContext(nc) as tc:
        with tc.tile_pool(name="const", bufs=1) as cpool, \
             tc.tile_pool(name="persist", bufs=1) as ppool:
            wqa = cpool.tile([128, 4, GD], F16)
            wqb = cpool.tile([128, 4, GD], F16)
            wk_sb = cpool.tile([128, NKT, GD], F16)
            wv_sb = cpool.tile([128, NKT, GD], F16)
            wo_sb = cpool.tile([128, GD // 128, D], F16)
            cos_sb = cpool.tile([128, S], F16)
            sin_sb = cpool.tile([128, S], F16)
            mask_sb = cpool.tile([128, 2, 2, 128], F16)
            warm = cpool.tile([128, 512], F16)
            nc.vector.memset(warm[:], 0.0)

            half = NKT * GD // 2
            # sync (SP HWDGE ring): wq first, then all x in chunk order
            nc.sync.dma_start(wqa[:].rearrange("p a b -> p (a b)"),
                              wq.ap()[:, 0:half])
            nc.sync.dma_start(wqb[:].rearrange("p a b -> p (a b)"),
                              wq.ap()[:, half:])
            xt = [[cpool.tile([128, 512], F16, name=f"x{kt}_{sc}")
                   for sc in range(NSC)] for kt in range(NKT)]
            for sc in range(NSC):
                for kt in range(NKT):
                    nc.sync.dma_start(
                        xt[kt][sc][:],
                        xT.ap()[:, kt * S + sc * 512:kt * S + sc * 512 + 512])
            # scalar (ACT HWDGE ring): k/v weights + tables + wo
            nc.scalar.dma_start(wk_sb[:].rearrange("p a b -> p (a b)"), wk.ap())
            nc.scalar.dma_start(wv_sb[:].rearrange("p a b -> p (a b)"), wv.ap())
            nc.scalar.dma_start(cos_sb[:], cos2[:])
            nc.scalar.dma_start(sin_sb[:], sin2[:])
            nc.scalar.dma_start(
                mask_sb[:].rearrange("p a b c -> p (a b c)"), mask4.ap())
            nc.scalar.dma_start(
                wo_sb[:].rearrange("p a b -> p (a b)"), wo.ap())

            # persistent activations; denominator ones column via memset
            v_sb = [ppool.tile([128, 4 * HPG * 65], F16, name=f"v{sc}")
                    for sc in range(NSC)]
            for sc in range(NSC):
                nc.gpsimd.memset(
                    v_sb[sc][:].rearrange("p (g c) -> p g c", c=65)[:, :, 64],
                    1.0)

            # preload the PartitionBroadcast Q7 library during the DMA
            # head so the first AV chain doesn't pay the ucode fetch
            warmb = ppool.tile([64, 8], F32, name="warmb")
            nc.gpsimd.partition_broadcast(warmb[:], warmb[0:1, :])

            qfc = [[ppool.tile([128, 512], F16, name=f"qf{t}_{c}")
                    for c in range(NSC)] for t in range(2)]
            kfc = [[ppool.tile([128, 512], F16, name=f"kf{t}_{c}")
                    for c in range(NSC)] for t in range(2)]
            yTq = [[ppool.tile([128, 512], F16, name=f"yT{t}_{g}")
                    for g in range(4)] for t in range(2)]

            with tc.tile_pool(name="attn", bufs=1) as apool, \
                 tc.tile_pool(name="smalls", bufs=4) as spool, \
                 tc.tile_pool(name="rope", bufs=3) as rawpool, \
                 tc.tile_pool(name="p3sb", bufs=4) as opool:
                attns = [[apool.tile([128, 768], F16, name=f"attn{th}_{kb}")
                          for kb in range(NKB)] for th in range(2)]
                u_cnt = [0]

                def emit_score(th, kb, scps):
                    q0 = kb * 128
                    n = min(384, S - q0)
                    sc_t = scps.tile([128, 1024], F32, tag="sc",
                                     name=f"sc{th}_{kb}")
                    c0, off0 = q0 // 512, q0 % 512
                    w0 = min(n, 512 - off0)
                    pieces = [(c0, off0, 0, w0)]
                    if w0 < n:
                        pieces.append((c0 + 1, 0, w0, n - w0))
                    for i in range(2):
                        ph = 64 * i
                        for (cc, coff, aoff, w) in pieces:
                            nc.tensor.matmul(
                                sc_t[:, i * 512 + aoff:i * 512 + aoff + w],
                                kfc[th][c0][ph:ph + 64, off0:off0 + 128],
                                qfc[th][cc][ph:ph + 64, coff:coff + w],
                                start=True, stop=True)
                    scv = sc_t[:].rearrange("p (g c) -> p g c", g=2)
                    at = attns[th][kb]
                    atv = at[:].rearrange("p (g c) -> p g c", g=2)
                    nc.scalar.activation(atv[:, :, 0:n], scv[:, :, 0:n],
                                         AF.Exp, scale=SCALE)
                    # multiplicative 0/1 mask on the triangle blocks
                    eng = nc.vector
                    u_cnt[0] += 1
                    if n == 384:
                        at4 = at[:].rearrange("p (g b c) -> p g b c",
                                              g=2, b=3)[:, :, 0::2, :]
                        eng.tensor_mul(at4, at4, mask_sb[:])
                    else:
                        eng.tensor_mul(atv[:, :, 0:128], atv[:, :, 0:128],
                                       mask_sb[:, :, 0, :])

                def rope_evac(acc, dst, s0, alt):
                    raw = rawpool.tile([128, 512], F16, tag="raw")
                    nc.scalar.copy(raw[:], acc[:])
                    rot = rawpool.tile([128, 512], F16, tag="rot")
                    nc.vector.stream_shuffle(rot[:], raw[:], SHUF_SWAP)
                    t1 = rawpool.tile([128, 512], F16, tag="t1")
                    nc.vector.tensor_mul(t1[:], rot[:],
                                         sin_sb[:, s0:s0 + 512])
                    t2 = rawpool.tile([128, 512], F16, tag="t2")
                    nc.vector.tensor_mul(t2[:], raw[:], cos_sb[:, s0:s0 + 512])
                    nc.vector.tensor_add(dst[:], t1[:], t2[:])

                # ---------------- phase A ----------------
                with tc.tile_pool(name="scpsA", bufs=2, space="PSUM") as scA:
                    with tc.tile_pool(name="p1acc", bufs=1,
                                      space="PSUM") as pps:
                        # PE warmup: ~3.4us of dummy matmuls so HAM reaches
                        # 8/8 by the time the first x/w tiles land
                        wacc = pps.tile([128, 512], F32, tag="acc0",
                                        name="warmacc")
                        for _ in range(8):
                            nc.tensor.matmul(wacc[:], warm[0:128, 0:128],
                                             warm[:], start=True, stop=True)

                        UM = {0: [], 1: [(0, 0), (1, 0)],
                              2: [(0, 2), (1, 2), (0, 3), (1, 3)],
                              3: [(0, 6), (1, 6), (0, 7), (1, 7)]}
                        UE = {0: [], 1: [(0, 1), (1, 1)],
                              2: [(0, 4), (1, 4), (0, 5), (1, 5)],
                              3: [(0, 8), (1, 8), (0, 9), (1, 9)]}

                        for sc in range(NSC):
                            s0 = sc * 512
                            accs = [pps.tile([128, 512], F32, tag=f"acc{t}",
                                             name=f"qk{sc}_{t}")
                                    for t in range(4)]
                            wsel = [(0, 0), (0, 128), (1, 0), (1, 128)]
                            for kt in range(NKT):
                                for t, (isk, off) in enumerate(wsel):
                                    if isk:
                                        w_t = wk_sb[:, kt, off:off + 128]
                                    elif kt < 4:
                                        w_t = wqa[:, kt, off:off + 128]
                                    else:
                                        w_t = wqb[:, kt - 4, off:off + 128]
                                    nc.tensor.matmul(
                                        accs[t][:], w_t, xt[kt][sc][:],
                                        start=(kt == 0), stop=(kt == NKT - 1))
                            rope_evac(accs[0], qfc[0][sc], s0, False)
                            rope_evac(accs[1], qfc[1][sc], s0, True)
                            rope_evac(accs[2], kfc[0][sc], s0, False)
                            rope_evac(accs[3], kfc[1][sc], s0, True)
                            for (th, kb) in UM[sc]:
                                emit_score(th, kb, scA)
                            # v-pass reuses banks 2-3 (k accs, freed by the
                            # rope PSUM copies)
                            av_ = [pps.tile([128, 512], F32, tag=f"acc{j+2}",
                                            name=f"v{sc}_{j}")
                                   for j in range(2)]
                            for kt in range(NKT):
                                st, sp = (kt == 0), (kt == NKT - 1)
                                for j in range(2):
                                    for jj in range(2):
                                        sb = 2 * j + jj
                                        nc.tensor.matmul(
                                            av_[j][:, jj * 256:(jj + 1) * 256],
                                            xt[kt][sc][:, sb * 128:
                                                       (sb + 1) * 128],
                                            wv_sb[:, kt, 0:256],
                                            start=(st and jj == 0),
                                            stop=(sp and jj == 1))
                            for j in range(2):
                                for jj in range(2):
                                    kbl = 2 * j + jj
                                    dstv = v_sb[sc][:, kbl * HPG * 65:
                                                    (kbl + 1) * HPG * 65]
                                    src = av_[j][:, jj * 256:(jj + 1) * 256] \
                                        .rearrange("p (g c) -> p g c", c=64)
                                    dv = dstv.rearrange("p (g c) -> p g c",
                                                        c=65)[:, :, 0:64]
                                    nc.scalar.copy(dv, src)
                            for (th, kb) in UE[sc]:
                                emit_score(th, kb, scA)

                # ---------------- post phase ----------------
                with tc.tile_pool(name="scpsB", bufs=2, space="PSUM") as scB, \
                     tc.tile_pool(name="avps", bufs=2, space="PSUM") as avps, \
                     tc.tile_pool(name="p3ps", bufs=2, space="PSUM") as ps3:

                    def emit_av(th, i, qq):
                        h = 2 * th + i
                        ph = 64 * i
                        acc = avps.tile([65, 512], F32, tag="av",
                                        name=f"av{th}_{i}_{qq}")
                        first = True
                        for j2 in range(2):      # 256-q halves
                            m = 2 * qq + j2
                            qb0 = 2 * m
                            mms = [(qb0, 0, 0, 256)]
                            if m >= 1:
                                mms.append((qb0 - 2, 0, 256, 128))
                                mms.append((qb0 - 1, 0, 128, 256))
                            mms.append((qb0 + 1, 128, 0, 128))
                            for ii, (kb, jo, ao, w) in enumerate(mms):
                                wdt = min(w, S - kb * 128 - ao)
                                nc.tensor.matmul(
                                    acc[:, j2 * 256 + jo:j2 * 256 + jo + wdt],
                                    v_sb[kb // 4][:, ((kb % 4) * HPG + h) * 65:
                                                  ((kb % 4) * HPG + h) * 65 + 65],
                                    attns[th][kb][:, i * 384 + ao:
                                                  i * 384 + ao + wdt],
                                    start=first,
                                    stop=(j2 == 1 and ii == len(mms) - 1))
                                first = False
                        # two immediate copies release the PSUM bank; the
                        # normalization chain then runs off SBUF
                        den = spool.tile([1, 512], F32, tag="den")
                        nc.scalar.copy(den[:], acc[64:65, :])
                        body = spool.tile([64, 512], F32, tag="body")
                        nc.vector.tensor_copy(body[:], acc[0:64, :])
                        rc0 = spool.tile([1, 512], F32, tag="rc0")
                        nc.vector.reciprocal_approx_fast(
                            out=rc0[:], in_=den[:])
                        # gpsimd runs ONLY partition_broadcast mid-kernel
                        # (single resident Q7 library: no ucode reloads)
                        rbs = spool.tile([64, 512], F32, tag="rbs")
                        nc.gpsimd.partition_broadcast(rbs[:], rc0[:])
                        nc.vector.tensor_mul(
                            yTq[th][qq][ph:ph + 64, :], body[:], rbs[:])

                    def emit_p3(stile):
                        r0 = stile * 128
                        g, o = stile // 4, (stile % 4) * 128
                        ot = opool.tile([128, D], F16, tag="ot")
                        for dc in range(2):
                            oacc = ps3.tile([128, 512], F32, tag="oacc")
                            for ct in range(2):
                                nc.tensor.matmul(
                                    oacc[:], yTq[ct][g][:, o:o + 128],
                                    wo_sb[:, ct, dc * 512:(dc + 1) * 512],
                                    start=(ct == 0), stop=(ct == 1))
                            if dc == 0:
                                nc.scalar.copy(ot[:, 0:512], oacc[:])
                            else:
                                nc.vector.tensor_copy(ot[:, 512:1024], oacc[:])
                        nc.sync.dma_start(out.ap()[r0:r0 + 128, :], ot[:])

                    # fully interleaved post phase: trail-score units (T),
                    # AV accs (A) and p3 stiles (P) alternate so no PSUM
                    # slot rotation ever head-blocks the PE queue
                    emit_score(0, 10, scB)
                    emit_av(0, 0, 0)
                    emit_score(1, 10, scB)
                    emit_av(0, 1, 0)
                    emit_score(0, 11, scB)
                    emit_av(1, 0, 0)
                    emit_score(1, 11, scB)
                    emit_av(1, 1, 0)
                    emit_score(0, 12, scB)
                    emit_av(0, 0, 1)
                    emit_score(1, 12, scB)
                    emit_av(0, 1, 1)
                    emit_score(0, 13, scB)
                    emit_av(1, 0, 1)
                    emit_score(1, 13, scB)
                    emit_av(1, 1, 1)
                    emit_p3(0)
                    emit_av(0, 0, 2)
                    emit_p3(1)
                    emit_av(0, 1, 2)
                    emit_score(0, 14, scB)
                    emit_av(1, 0, 2)
                    emit_score(1, 14, scB)
                    emit_av(1, 1, 2)
                    emit_p3(2)
                    emit_score(0, 15, scB)
                    emit_p3(3)
                    emit_score(1, 15, scB)
                    emit_av(0, 0, 3)
                    emit_p3(4)
                    emit_av(0, 1, 3)
                    emit_p3(5)
                    emit_av(1, 0, 3)
                    emit_p3(6)
                    emit_av(1, 1, 3)
                    for stile in range(7, 16):
                        emit_p3(stile)

    nc.finalize()
    return nc


def _rope_tables():
    inv_freq = 1.0 / (THETA ** (np.arange(0, HD, 2, dtype=np.float64) / HD))
    t = np.arange(S, dtype=np.float64) / max(SCALING, 1e-6)
    freqs = np.outer(t, inv_freq)                      # [S, HD/2]
    emb = np.concatenate((freqs, freqs), axis=-1)      # [S, HD]
    return np.cos(emb), np.sin(emb)


def _swz(w):
    # [kt*128, X] -> [128, kt*X] partition-major contiguous
    kt = w.shape[0] // 128
    return np.ascontiguousarray(
        w.reshape(kt, 128, w.shape[1]).transpose(1, 0, 2).reshape(128, -1))


def _host_prep(x, Wq, Wk, Wv, Wo):
    cos, sin = _rope_tables()
    cosT2 = np.ascontiguousarray(np.tile(cos.T, (2, 1))).astype(np.float16)
    sinT2 = np.ascontiguousarray(np.tile(sin.T, (2, 1))).astype(np.float16)
    # fold the rotate-half signs into the sin table: rot[2i] = -raw[2i+1],
    # rot[2i+1] = +raw[2i]; the shuffle moves values unsigned
    sinT2[0::2, :] *= -1.0

    # mask4 [128 key, (i=2, block=2, 128 col)]: block 0 = causal triangle of
    # the kb-aligned window block, block 1 = far-window triangle (col+256)
    cc = np.arange(128)[None, :]
    kk = np.arange(128)[:, None]
    mb0 = (cc >= kk).astype(np.float16)        # [128, 128]
    mb1 = (cc < kk).astype(np.float16)
    mi = np.concatenate([mb0, mb1], axis=1)    # [128, 256]
    m4 = np.ascontiguousarray(np.concatenate([mi, mi], axis=1))  # [128, 512]

    in_maps = []
    for c in range(8):
        b, g = c // HG, c % HG
        gsl = slice(g * GD, (g + 1) * GD)
        in_maps.append({
            "xT": _swz(x[b].T.astype(np.float16).reshape(D, S)),
            "wq": _swz(Wq[gsl, :].T).astype(np.float16),
            "wk": _swz(Wk[gsl, :].T).astype(np.float16),
            "wv": _swz(Wv[gsl, :].T).astype(np.float16),
            "wo": _swz(Wo[:, gsl].T).astype(np.float16),
            "cos2": cosT2, "sin2": sinT2, "mask4": m4,
        })
    return in_maps


def _run(inputs, trace=False, **kw):
    if "nc" not in _CACHE:
        _CACHE["nc"] = _build()
    in_maps = _host_prep(inputs["x"], inputs["Wq"], inputs["Wk"],
                         inputs["Wv"], inputs["Wo"])
    return run_bass_kernel_spmd(_CACHE["nc"], in_maps, list(range(8)),
                                trace=trace, **kw)


def kernel(x, Wq, Wk, Wv, Wo):
    res = _run({"x": x, "Wq": Wq, "Wk": Wk, "Wv": Wv, "Wo": Wo})
    out = np.zeros((B, S, D), dtype=np.float32)
    for c in range(8):
        out[c // HG] += res.results[c]["out"].astype(np.float32)
    return out
